# revision 1
# baseline (speedup 1.0000x reference)
"""GAT layer kernel for Trainium2, 8 NeuronCores, edge/node-parallel.

Strategy (dst-sorted node sharding):
  - Sort edges by dst; partition nodes into 8 contiguous ranges with ~E/8
    edges each.  Each core owns its dst-range's nodes and all their in-edges.
  - Node phase (replicated): LayerNorm stats for all nodes via PE matmuls on a
    host-pretransposed h^T, packed finish on DVE/ACT -> eh, et; build a DRAM
    table T3[n] = [h[n] | 1.0 | eh[n] | et[n] | pad] (192 f32 = 768 B rows).
  - Edge phase per 128-dst block: er = tanh(LN(r)@w) from a host-pretransposed
    r^T stream (PE stats matmuls + batched row finish), dma_gather of T3 rows
    by src (768 B) and of the scalar region by dst (256 B), softmax without
    max-subtraction (e in [0,3) so exp is safe; exp(relu(x)) == max(1,exp(x))),
    scaled one-hot built in ONE tensor_scalar op, and a PSUM-accumulated
    matmul onehot^T @ [h|1] that yields feat and esum together.
  - Final: feat/esum, feat @ fc_w + b, row L2 normalize, DMA out.
"""

import os
import sys

sys.path.insert(0, "/opt/trn_rl_repo")

_PHASES = int(os.environ.get("KPHASES", "4"))

import numpy as np

import concourse.bacc as bacc
import concourse.bass as bass
import concourse.mybir as mybir
import concourse.tile as tile
from concourse.bass_interp import get_hw_module

F32 = mybir.dt.float32
I16 = mybir.dt.int16
AF = mybir.ActivationFunctionType
OP = mybir.AluOpType

N = 20000
E = 640000
D = 128
NCORES = 8
EPS = 1e-6
NPAD = 20480          # nodes padded to 40*512
NB_U = 21             # uniform blocks (of 128 dst nodes) per core
TROW = 192            # T3 table row: [h(128) | 1 | eh | et | pad] f32 (768B)
SCOFF = 128           # scalar region offset in T3 row
SCW = 64              # scalar region width (256B)


# ----------------------------------------------------------------- host prep
def _host_prep(h, r, src, dst, hn_a, hn_b, tn_a, tn_b, rn_a, rn_b,
               head_w, tail_w, rel_w, fc_w, fc_b):
    h = np.asarray(h, np.float32); r = np.asarray(r, np.float32)
    src = np.asarray(src, np.int32); dst = np.asarray(dst, np.int32)

    u_h = np.asarray(hn_a, np.float32) * np.asarray(head_w, np.float32)
    u_t = np.asarray(tn_a, np.float32) * np.asarray(tail_w, np.float32)
    u_r = np.asarray(rn_a, np.float32) * np.asarray(rel_w, np.float32)
    s_uh = float(u_h.sum()); s_ut = float(u_t.sum()); s_ur = float(u_r.sum())
    c_h = float((np.asarray(hn_b, np.float32) * head_w).sum())
    c_t = float((np.asarray(tn_b, np.float32) * tail_w).sum())
    c_r = float((np.asarray(rn_b, np.float32) * rel_w).sum())

    perm = np.argsort(dst, kind="stable")
    dst_s = dst[perm]; src_s = src[perm]

    counts = np.bincount(dst, minlength=N)
    cum = np.concatenate([[0], np.cumsum(counts)])

    # node range boundaries: ~E/8 edges each, capped at NB_U*128 nodes
    bounds = [0]
    for k in range(1, NCORES):
        n = int(np.searchsorted(cum, k * E / NCORES))
        n = max(bounds[-1] + 1, min(n, bounds[-1] + NB_U * 128))
        n = max(n, N - (NCORES - k) * NB_U * 128)   # leave room for the rest
        bounds.append(n)
    bounds.append(N)

    # T_B: max tiles over every (core, block)
    t_b = 1
    for k in range(NCORES):
        nlo, nhi = bounds[k], bounds[k + 1]
        for b0 in range(nlo, nhi, 128):
            cnt = int(cum[min(b0 + 128, nhi)] - cum[b0])
            t_b = max(t_b, (cnt + 127) // 128)
    e_blk = t_b * 128
    s_b = e_blk // 16
    ep = NB_U * e_blk

    # replicated tensors
    h_pad = np.empty((NPAD, D), np.float32)
    h_pad[:N] = h; h_pad[N:] = h[0]
    hT = np.ascontiguousarray(h_pad.T)
    iota = np.broadcast_to(np.arange(128, dtype=np.float32), (128, 128)).copy()
    ident = np.eye(128, dtype=np.float32)
    wn = np.zeros((128, 4), np.float32)
    wn[:, 0] = 1.0; wn[:, 1] = u_h; wn[:, 2] = u_t
    wr = np.zeros((128, 2), np.float32)
    wr[:, 0] = 1.0; wr[:, 1] = u_r
    fcw = np.ascontiguousarray(np.asarray(fc_w, np.float32))
    fcb = np.broadcast_to(np.asarray(fc_b, np.float32), (128, 128)).copy()
    consts = np.zeros((128, 8), np.float32)
    consts[:, 0] = s_uh; consts[:, 1] = s_ut; consts[:, 2] = s_ur
    consts[:, 3] = c_h; consts[:, 4] = c_t; consts[:, 5] = c_r

    rep = {"hT": hT, "h_nat": h_pad, "iota": iota, "ident": ident,
           "wn": wn, "wr": wr, "fcw": fcw, "fcb": fcb, "consts": consts}

    in_maps = []
    for k in range(NCORES):
        nlo, nhi = bounds[k], bounds[k + 1]
        nb = (nhi - nlo + 127) // 128
        # per-slot arrays, one row of NB_U*e_blk slots
        src16 = np.zeros((NB_U, e_blk), np.int16)
        dst16 = np.zeros((NB_U, e_blk), np.int16)
        dstl = np.zeros((NB_U, e_blk), np.float32)
        valid = np.zeros((NB_U, e_blk), np.float32)
        rcol = np.zeros((NB_U, e_blk), np.int64)
        for b in range(nb):
            b0 = nlo + 128 * b
            e0, e1 = int(cum[b0]), int(cum[min(b0 + 128, nhi)])
            cnt = e1 - e0
            src16[b, :cnt] = src_s[e0:e1]
            dst16[b, :cnt] = dst_s[e0:e1]
            dstl[b, :cnt] = (dst_s[e0:e1] - b0).astype(np.float32)
            valid[b, :cnt] = 1.0
            rcol[b, :cnt] = perm[e0:e1]
        # rT: [128, ep], column (b*e_blk + j) = r[rcol]
        rT = np.ascontiguousarray(r[rcol.reshape(-1)].T)
        # idx tensors: per block wrap 16, replicate x8 -> [128, NB_U*s_b]
        def wrap16(a):
            blk = a.reshape(NB_U, s_b, 16).transpose(0, 2, 1)  # [NB_U,16,s_b]
            out = np.tile(blk, (1, 8, 1))                       # [NB_U,128,s_b]
            return np.ascontiguousarray(out.transpose(1, 0, 2).reshape(128, NB_U * s_b))
        idx_src = wrap16(src16)
        idx_dst = wrap16(dst16)
        # packed [128, NB_U*t_b]: [p, b*t_b+t] = slot j=128t+p
        def pk(a):
            x = a.reshape(NB_U, t_b, 128).transpose(2, 0, 1)   # [128, NB_U, t_b]
            return np.ascontiguousarray(x.reshape(128, NB_U * t_b))
        in_maps.append(dict(rep, rT=rT, idx_src=idx_src, idx_dst=idx_dst,
                            dstl=pk(dstl), valid=pk(valid)))
    meta = dict(t_b=t_b, e_blk=e_blk, s_b=s_b, ep=ep, bounds=bounds)
    return in_maps, meta


# ------------------------------------------------------------ device program
def build_program(t_b, loop_k=1, for_hw=True):
    e_blk = t_b * 128
    s_b = e_blk // 16
    ep = NB_U * e_blk
    nc = bacc.Bacc("TRN2", target_bir_lowering=False, debug=False,
                   enable_asserts=False, num_devices=NCORES if for_hw else 1)

    dt_rT = nc.dram_tensor("rT", [128, ep], F32, kind="ExternalInput")
    dt_hT = nc.dram_tensor("hT", [128, NPAD], F32, kind="ExternalInput")
    dt_hn = nc.dram_tensor("h_nat", [NPAD, D], F32, kind="ExternalInput")
    dt_isrc = nc.dram_tensor("idx_src", [128, NB_U * s_b], I16, kind="ExternalInput")
    dt_idst = nc.dram_tensor("idx_dst", [128, NB_U * s_b], I16, kind="ExternalInput")
    dt_dstl = nc.dram_tensor("dstl", [128, NB_U * t_b], F32, kind="ExternalInput")
    dt_valid = nc.dram_tensor("valid", [128, NB_U * t_b], F32, kind="ExternalInput")
    dt_iota = nc.dram_tensor("iota", [128, 128], F32, kind="ExternalInput")
    dt_ident = nc.dram_tensor("ident", [128, 128], F32, kind="ExternalInput")
    dt_wn = nc.dram_tensor("wn", [128, 4], F32, kind="ExternalInput")
    dt_wr = nc.dram_tensor("wr", [128, 2], F32, kind="ExternalInput")
    dt_fcw = nc.dram_tensor("fcw", [128, 128], F32, kind="ExternalInput")
    dt_fcb = nc.dram_tensor("fcb", [128, 128], F32, kind="ExternalInput")
    dt_consts = nc.dram_tensor("consts", [128, 8], F32, kind="ExternalInput")
    dt_out = nc.dram_tensor("out", [NB_U * 128, 128], F32, kind="ExternalOutput")
    dt_T3 = nc.dram_tensor("T3", [NPAD, TROW], F32, kind="ExternalOutput")

    NG = NPAD // 512          # node-phase groups
    NPK = NPAD // 128         # packed node cols
    EPK = NB_U * t_b          # packed edge cols
    GE = (e_blk + 511) // 512  # stats groups per block

    with tile.TileContext(nc) as tc:
        with tc.tile_pool(name="const", bufs=1) as cpool:
            iota_sb = cpool.tile([128, 128], F32)
            nc.sync.dma_start(out=iota_sb[:], in_=dt_iota.ap())
            ident_sb = cpool.tile([128, 128], F32)
            nc.sync.dma_start(out=ident_sb[:], in_=dt_ident.ap())
            wn_sb = cpool.tile([128, 4], F32)
            nc.sync.dma_start(out=wn_sb[:], in_=dt_wn.ap())
            wr_sb = cpool.tile([128, 2], F32)
            nc.sync.dma_start(out=wr_sb[:], in_=dt_wr.ap())
            fcw_sb = cpool.tile([128, 128], F32)
            nc.sync.dma_start(out=fcw_sb[:], in_=dt_fcw.ap())
            fcb_sb = cpool.tile([128, 128], F32)
            nc.sync.dma_start(out=fcb_sb[:], in_=dt_fcb.ap())
            cst = cpool.tile([128, 8], F32)
            nc.sync.dma_start(out=cst[:], in_=dt_consts.ap())
            isrc_sb = cpool.tile([128, NB_U * s_b], I16)
            nc.sync.dma_start(out=isrc_sb[:], in_=dt_isrc.ap())
            idst_sb = cpool.tile([128, NB_U * s_b], I16)
            nc.sync.dma_start(out=idst_sb[:], in_=dt_idst.ap())
            dstl_sb = cpool.tile([128, NB_U * t_b], F32)
            nc.sync.dma_start(out=dstl_sb[:], in_=dt_dstl.ap())
            valid_sb = cpool.tile([128, NB_U * t_b], F32)
            nc.sync.dma_start(out=valid_sb[:], in_=dt_valid.ap())

            def loop_body():
                # ======================= node phase: stats =======================
                # per 128-node tile: mm(lhsT=hT_slice [D,128n], rhs=wn [D,3])
                # -> psum [128n, 3] already packed; s2 via squared lhsT, N=1.
                with tc.tile_pool(name="nstat", bufs=1) as spool, \
                     tc.tile_pool(name="nwork", bufs=3) as wpool, \
                     tc.tile_pool(name="npsum", bufs=4, space="PSUM") as pp:
                    spk = spool.tile([128, NPK, 4], F32)
                    ehp = spool.tile([128, NPK], F32)
                    etp = spool.tile([128, NPK], F32)
                    for g in range(NG):
                        hTg = wpool.tile([128, 512], F32, tag="hTg")
                        nc.sync.dma_start(out=hTg[:], in_=dt_hT.ap()[:, 512 * g:512 * (g + 1)])
                        psS = pp.tile([128, 16], F32, tag="psS")
                        for c in range(4):
                            nc.tensor.matmul(psS[:, 4 * c:4 * c + 3],
                                             hTg[:, 128 * c:128 * (c + 1)],
                                             wn_sb[:, 0:3], start=True, stop=True)
                        nc.scalar.activation(out=hTg[:], in_=hTg[:], func=AF.Square)
                        for c in range(4):
                            nc.tensor.matmul(psS[:, 4 * c + 3:4 * c + 4],
                                             hTg[:, 128 * c:128 * (c + 1)],
                                             wn_sb[:, 0:1], start=True, stop=True)
                        nc.scalar.activation(out=spk[:, 4 * g:4 * (g + 1), :], in_=psS[:], func=AF.Copy)
                    # batched finish -> eh, et (strided stat views)
                    s1p = spk[:, :, 0]; suh = spk[:, :, 1]
                    sut = spk[:, :, 2]; s2p = spk[:, :, 3]
                    mu = spool.tile([128, NPK], F32)
                    nc.vector.tensor_scalar_mul(out=mu[:], in0=s1p, scalar1=1.0 / 128.0)
                    t0 = spool.tile([128, NPK], F32)
                    nc.vector.tensor_mul(out=t0[:], in0=mu[:], in1=mu[:])
                    nc.vector.tensor_scalar_mul(out=t0[:], in0=t0[:], scalar1=-128.0)
                    nc.vector.tensor_add(out=t0[:], in0=t0[:], in1=s2p)
                    rstd = spool.tile([128, NPK], F32)
                    nc.scalar.activation(out=rstd[:], in_=t0[:], func=AF.Sqrt, scale=1.0 / 127.0)
                    nc.vector.tensor_scalar_add(out=rstd[:], in0=rstd[:], scalar1=EPS)
                    nc.vector.reciprocal(out=rstd[:], in_=rstd[:])
                    for su, sidx, cidx, dest in ((suh, 0, 3, ehp), (sut, 1, 4, etp)):
                        m1 = spool.tile([128, NPK], F32, tag="m1")
                        nc.vector.tensor_scalar_mul(out=m1[:], in0=mu[:], scalar1=cst[:, sidx:sidx + 1])
                        nc.vector.tensor_sub(out=m1[:], in0=su, in1=m1[:])
                        nc.vector.tensor_mul(out=m1[:], in0=m1[:], in1=rstd[:])
                        nc.vector.tensor_scalar_add(out=m1[:], in0=m1[:], scalar1=cst[:, cidx:cidx + 1])
                        nc.scalar.activation(out=dest[:], in_=m1[:], func=AF.Tanh)
                    if _PHASES == 1:
                        nc.sync.dma_start(out=dt_out.ap()[0:128, 0:min(NPK, 128)],
                                          in_=ehp[:, 0:min(NPK, 128)])
                        return
                    # ====================== T3 table build ======================
                    with tc.tile_pool(name="tbld", bufs=3) as tb_pool:
                        for g in range(NG):
                            tb = tb_pool.tile([128, 4, TROW], F32, tag="tb")
                            nc.sync.dma_start(
                                out=tb[:, :, 0:D],
                                in_=dt_hn.ap()[512 * g:512 * (g + 1), :]
                                    .rearrange("(c p) d -> p c d", p=128))
                            nc.vector.memset(tb[:, :, SCOFF:SCOFF + 1], 1.0)
                            nc.vector.tensor_copy(out=tb[:, :, SCOFF + 1], in_=ehp[:, 4 * g:4 * (g + 1)])
                            nc.vector.tensor_copy(out=tb[:, :, SCOFF + 2], in_=etp[:, 4 * g:4 * (g + 1)])
                            nc.vector.memset(tb[:, :, SCOFF + 3:TROW], 0.0)
                            nc.sync.dma_start(
                                out=dt_T3.ap()[512 * g:512 * (g + 1), :]
                                    .rearrange("(c p) w -> p c w", p=128),
                                in_=tb[:, :, :])

                # ===================== edge phase 1: er =========================
                with tc.tile_pool(name="estat", bufs=1) as espool:
                    epk3 = espool.tile([128, EPK, 3], F32)
                    erp = espool.tile([128, EPK], F32)
                    with tc.tile_pool(name="ework", bufs=2) as ewpool, \
                         tc.tile_pool(name="epsum", bufs=2, space="PSUM") as epp:
                        for b in range(NB_U):
                            rTb = ewpool.tile([128, e_blk], F32, tag="rTb")
                            nc.sync.dma_start(out=rTb[:], in_=dt_rT.ap()[:, b * e_blk:(b + 1) * e_blk])
                            psE = epp.tile([128, 3 * t_b], F32, tag="psE")
                            for t in range(t_b):
                                nc.tensor.matmul(psE[:, 3 * t:3 * t + 2],
                                                 rTb[:, 128 * t:128 * (t + 1)],
                                                 wr_sb[:], start=True, stop=True)
                            nc.scalar.activation(out=rTb[:], in_=rTb[:], func=AF.Square)
                            for t in range(t_b):
                                nc.tensor.matmul(psE[:, 3 * t + 2:3 * t + 3],
                                                 rTb[:, 128 * t:128 * (t + 1)],
                                                 wr_sb[:, 0:1], start=True, stop=True)
                            nc.scalar.activation(out=epk3[:, b * t_b:(b + 1) * t_b, :],
                                                 in_=psE[:], func=AF.Copy)
                    # batched er finish (strided stat views)
                    s1e = epk3[:, :, 0]; sue = epk3[:, :, 1]; s2e = epk3[:, :, 2]
                    mu = espool.tile([128, EPK], F32)
                    nc.vector.tensor_scalar_mul(out=mu[:], in0=s1e, scalar1=1.0 / 128.0)
                    t0 = espool.tile([128, EPK], F32)
                    nc.vector.tensor_mul(out=t0[:], in0=mu[:], in1=mu[:])
                    nc.vector.tensor_scalar_mul(out=t0[:], in0=t0[:], scalar1=-128.0)
                    nc.vector.tensor_add(out=t0[:], in0=t0[:], in1=s2e)
                    rstd = espool.tile([128, EPK], F32)
                    nc.scalar.activation(out=rstd[:], in_=t0[:], func=AF.Sqrt, scale=1.0 / 127.0)
                    nc.vector.tensor_scalar_add(out=rstd[:], in0=rstd[:], scalar1=EPS)
                    nc.vector.reciprocal(out=rstd[:], in_=rstd[:])
                    m1 = espool.tile([128, EPK], F32)
                    nc.vector.tensor_scalar_mul(out=m1[:], in0=mu[:], scalar1=cst[:, 2:3])
                    nc.vector.tensor_sub(out=m1[:], in0=sue, in1=m1[:])
                    nc.vector.tensor_mul(out=m1[:], in0=m1[:], in1=rstd[:])
                    nc.vector.tensor_scalar_add(out=m1[:], in0=m1[:], scalar1=cst[:, 5:6])
                    nc.scalar.activation(out=erp[:], in_=m1[:], func=AF.Tanh)
                    if _PHASES == 2:
                        nc.sync.dma_start(out=dt_out.ap()[0:128, 0:min(EPK, 128)],
                                          in_=erp[:, 0:min(EPK, 128)])
                        return

                    # ================= edge phase 2: gather + feat ==============
                    with tc.tile_pool(name="gwork", bufs=2) as gpool, \
                         tc.tile_pool(name="feat", bufs=1) as fpool, \
                         tc.tile_pool(name="fpsum", bufs=2, space="PSUM") as fpp:
                        featst = fpool.tile([128, NB_U * 129], F32)
                        for b in range(NB_U):
                            tg = gpool.tile([128, t_b, TROW], F32, tag="tg")
                            nc.gpsimd.dma_gather(
                                out_ap=tg[:, :, :], in_ap=dt_T3.ap(),
                                idxs_ap=isrc_sb[:, b * s_b:(b + 1) * s_b],
                                num_idxs=e_blk, num_idxs_reg=e_blk, elem_size=TROW,
                                single_packet=False)
                            sc = gpool.tile([128, t_b, SCW], F32, tag="sc")
                            nc.gpsimd.dma_gather(
                                out_ap=sc[:, :, :], in_ap=dt_T3.ap()[:, SCOFF:TROW],
                                idxs_ap=idst_sb[:, b * s_b:(b + 1) * s_b],
                                num_idxs=e_blk, num_idxs_reg=e_blk,
                                elem_size=SCW, elem_step=TROW, single_packet=False)
                            # exe = max(1, exp(eh_src + et_dst + er)) * valid
                            ex = gpool.tile([128, t_b], F32, tag="ex")
                            nc.vector.tensor_tensor(out=ex[:], in0=tg[:, :, SCOFF + 1],
                                                    in1=sc[:, :, 2], op=OP.add)
                            nc.vector.tensor_add(out=ex[:], in0=ex[:],
                                                 in1=erp[:, b * t_b:(b + 1) * t_b])
                            nc.scalar.activation(out=ex[:], in_=ex[:], func=AF.Exp)
                            nc.vector.tensor_scalar(out=ex[:], in0=ex[:],
                                                    scalar1=1.0, scalar2=None,
                                                    op0=OP.max)
                            nc.vector.tensor_mul(out=ex[:], in0=ex[:],
                                                 in1=valid_sb[:, b * t_b:(b + 1) * t_b])
                            psF = fpp.tile([128, 129], F32, tag="psF")
                            for t in range(t_b):
                                oh = gpool.tile([128, 128], F32, tag="oh")
                                nc.vector.tensor_scalar(
                                    out=oh[:], in0=iota_sb[:],
                                    scalar1=dstl_sb[:, b * t_b + t:b * t_b + t + 1],
                                    scalar2=ex[:, t:t + 1],
                                    op0=OP.is_equal, op1=OP.mult)
                                nc.tensor.matmul(psF[:], oh[:], tg[:, t, 0:129],
                                                 start=(t == 0), stop=(t == t_b - 1))
                            nc.scalar.activation(out=featst[:, b * 129:(b + 1) * 129],
                                                 in_=psF[:], func=AF.Copy)
                        if _PHASES == 3:
                            nc.sync.dma_start(out=dt_out.ap()[0:128, 0:128],
                                              in_=featst[:, 0:128])
                            return
                        # =================== final per block ====================
                        with tc.tile_pool(name="fin", bufs=2) as npool, \
                             tc.tile_pool(name="finps", bufs=2, space="PSUM") as npp:
                            for b in range(NB_U):
                                rs = npool.tile([128, 1], F32, tag="rs")
                                nc.vector.tensor_scalar(out=rs[:], in0=featst[:, b * 129 + 128:b * 129 + 129],
                                                        scalar1=1e-30, scalar2=None, op0=OP.max)
                                nc.vector.reciprocal(out=rs[:], in_=rs[:])
                                fs = npool.tile([128, 128], F32, tag="fs")
                                nc.vector.tensor_scalar_mul(
                                    out=fs[:], in0=featst[:, b * 129:b * 129 + 128], scalar1=rs[:])
                                if _PHASES == 5:
                                    nc.sync.dma_start(out=dt_out.ap()[b * 128:(b + 1) * 128, :], in_=fs[:])
                                    continue
                                psT = npp.tile([128, 128], F32, tag="psT")
                                nc.tensor.transpose(psT[:], fs[:], ident_sb[:])
                                fT = npool.tile([128, 128], F32, tag="fT")
                                nc.scalar.activation(out=fT[:], in_=psT[:], func=AF.Copy)
                                if _PHASES == 6:
                                    nc.sync.dma_start(out=dt_out.ap()[b * 128:(b + 1) * 128, :], in_=fT[:])
                                    continue
                                psO = npp.tile([128, 128], F32, tag="psO")
                                nc.tensor.matmul(psO[:], fT[:], fcw_sb[:], start=True, stop=True)
                                ob = npool.tile([128, 128], F32, tag="ob")
                                nc.vector.tensor_add(out=ob[:], in0=psO[:], in1=fcb_sb[:])
                                if _PHASES == 7:
                                    nc.sync.dma_start(out=dt_out.ap()[b * 128:(b + 1) * 128, :], in_=ob[:])
                                    continue
                                scr = npool.tile([128, 128], F32, tag="scr")
                                nrm = npool.tile([128, 1], F32, tag="nrm")
                                nc.vector.tensor_mul(out=scr[:], in0=ob[:], in1=ob[:])
                                nc.vector.reduce_sum(out=nrm[:], in_=scr[:],
                                                     axis=mybir.AxisListType.X)
                                if _PHASES == 8:
                                    nc.sync.dma_start(out=dt_out.ap()[b * 128:(b + 1) * 128, :], in_=scr[:])
                                    continue
                                nc.scalar.activation(out=nrm[:], in_=nrm[:], func=AF.Sqrt)
                                if _PHASES == 9:
                                    nc.sync.dma_start(out=dt_out.ap()[b * 128:(b + 1) * 128, 0:1], in_=nrm[:])
                                    continue
                                nc.vector.tensor_scalar(out=nrm[:], in0=nrm[:],
                                                        scalar1=1e-12, scalar2=None, op0=OP.max)
                                nc.vector.reciprocal(out=nrm[:], in_=nrm[:])
                                nc.vector.tensor_scalar_mul(out=ob[:], in0=ob[:], scalar1=nrm[:])
                                nc.sync.dma_start(out=dt_out.ap()[b * 128:(b + 1) * 128, :], in_=ob[:])

            if loop_k == 1:
                loop_body()
            else:
                with tc.For_i(0, loop_k, 1):
                    loop_body()

    nc.compile()
    if for_hw:
        nc.m = get_hw_module(nc.m)
    return nc


# ------------------------------------------------------------------- runner
class Runner:
    def __init__(self, nc, n_cores=NCORES):
        import jax
        from concourse.bass2jax import (_bass_exec_p, partition_id_tensor,
                                        install_neuronx_cc_hook)
        from jax.sharding import Mesh, PartitionSpec, NamedSharding
        from jax.experimental.shard_map import shard_map
        install_neuronx_cc_hook()
        self.jax = jax
        self.n_cores = n_cores
        pname = nc.partition_id_tensor.name if nc.partition_id_tensor else None
        in_names, out_names, out_avals = [], [], []
        for alloc in nc.m.functions[0].allocations:
            if not isinstance(alloc, mybir.MemoryLocationSet):
                continue
            name = alloc.memorylocations[0].name
            if alloc.kind == "ExternalInput":
                if name != pname:
                    in_names.append(name)
            elif alloc.kind == "ExternalOutput":
                out_names.append(name)
                out_avals.append(jax.core.ShapedArray(
                    tuple(alloc.tensor_shape), mybir.dt.np(alloc.dtype)))
        self.in_names, self.out_names, self.out_avals = in_names, out_names, out_avals
        n_params = len(in_names)
        all_in = list(in_names) + list(out_names)
        if pname is not None:
            all_in.append(pname)

        def _body(*args):
            operands = list(args)
            if pname is not None:
                operands.append(partition_id_tensor())
            return tuple(_bass_exec_p.bind(
                *operands, out_avals=tuple(out_avals), in_names=tuple(all_in),
                out_names=tuple(out_names), lowering_input_output_aliases=(),
                sim_require_finite=True, sim_require_nnan=True, nc=nc))

        devices = jax.devices()[:n_cores]
        self.mesh = Mesh(np.asarray(devices), ("core",))
        self.sharding = NamedSharding(self.mesh, PartitionSpec("core"))
        donate = tuple(range(n_params, n_params + len(out_names)))
        self.fn = jax.jit(shard_map(
            _body, mesh=self.mesh,
            in_specs=(PartitionSpec("core"),) * (n_params + len(out_names)),
            out_specs=(PartitionSpec("core"),) * len(out_names),
            check_rep=False), donate_argnums=donate, keep_unused=True)

    def put_inputs(self, in_maps):
        return [self.jax.device_put(
            np.concatenate([np.asarray(in_maps[c][nm]) for c in range(self.n_cores)], axis=0),
            self.sharding) for nm in self.in_names]

    def put_zeros(self):
        return [self.jax.device_put(
            np.zeros((self.n_cores * a.shape[0], *a.shape[1:]), a.dtype), self.sharding)
            for a in self.out_avals]

    def run(self, dev_in, dev_zeros):
        outs = self.fn(*dev_in, *dev_zeros)
        self.jax.block_until_ready(outs)
        return outs

    def unpack(self, outs):
        return [{nm: np.asarray(outs[i]).reshape(self.n_cores, *self.out_avals[i].shape)[c]
                 for i, nm in enumerate(self.out_names)} for c in range(self.n_cores)]


_CACHE = {}


def _get_runner(t_b, loop_k=1):
    key = (t_b, loop_k)
    if key not in _CACHE:
        nc = build_program(t_b, loop_k)
        _CACHE[key] = Runner(nc)
    return _CACHE[key]


def kernel(**inputs):
    in_maps, meta = _host_prep(**inputs)
    r = _get_runner(meta["t_b"], 1)
    dev = r.put_inputs(in_maps)
    res = r.unpack(r.run(dev, r.put_zeros()))
    bounds = meta["bounds"]
    out = np.empty((N, D), np.float32)
    for k in range(NCORES):
        nlo, nhi = bounds[k], bounds[k + 1]
        out[nlo:nhi] = res[k]["out"][:nhi - nlo]
    return out

